# revision 1
# baseline (speedup 1.0000x reference)
"""Trainium2 Bass kernel for the DNA GNN (nn_DNA_65360812310552).

Strategy (8 NeuronCores, SPMD):
  - Nodes padded to NP=10240, sharded by col-range: core c owns nodes
    [c*1280, (c+1)*1280) and ALL edges whose target (col) lies in that
    range.  Aggregation is core-local: no reduce collectives.
  - All static graph data is HOST-precomputed: deg/dis (gcn norm), the
    per-tile segment-sum selection matrices S'' (bf16, with
    dis[row]*dis[col] folded in, SBUF-resident), and the bias-path
    vector st' = dis_i * segsum(dis[row]).  The device does no prep
    passes and no dis AllGather.
  - Algebra: bk cancels in softmax; Wk is folded into the query
    (qt = glinT(glin(x,Wq)+bq, Wk)/sqrt(CH)); Wv+bv are deferred past
    attention + segment-sum (linearity).
  - Edge phase per 128-node window, in chunks of <=NB 128-edge tiles,
    with ALL layer-slices batched per DVE op:
      P   = xga * qg            (bf16, 2x mode)
      sc  = tree-reduce_CH(P)   (3 bf16 stages + 1 f32, strided out)
      ex  = exp(sc)             (ACT)
      den/rec/attn              (small DVE ops)
      EXB = broadcast_CH(attn)  (ACT, bf16 out)
      MSG = xga * EXB           (bf16, 2x mode)
      psum[window] += S''_t^T @ MSG[t,j]   (PE, S'' stationary per tile)
  - Node-table AllGather per layer is split in two halves so the first
    half overlaps the last 5 windows' compute; gather row indices are
    host-remapped to the split layout.

Self-contained: hardcodes shapes; builds the Bass program per input
(edge partition sizes baked in), runs via run_bass_kernel_spmd on
cores 0-7, reassembles the full [10000, 16] output.
"""

import numpy as np

import concourse.bacc as bacc
import concourse.bass as bass
import concourse.mybir as mybir
import concourse.tile as tile
from concourse.bass_utils import run_bass_kernel_spmd
from concourse.masks import make_identity

# problem constants
N = 10000
E = 160000
C = 128
H = 8
CH = 16
G = 16
CG = 8
L = 5
NF = 14
NFP = 16          # NF padded
DOUT = 16
NCORES = 8

NP = 10240        # padded node count = 8 * 1280
NSL = NP // NCORES  # 1280 nodes per core
NW = NSL // 128     # 10 windows of 128 nodes per core
NHALF = NSL // 2    # AG split point (640 = windows 0-4)
NB = 8              # max tiles (of 128 edges) per chunk

F32 = mybir.dt.float32
BF16 = mybir.dt.bfloat16
I16 = mybir.dt.int16

EDT = mybir.dt.bfloat16   # edge-pipeline dtype


def _wrap_idx(a: np.ndarray) -> np.ndarray:
    """[T] int -> [128, T//16] int16 in dma_gather's wrapped layout:
    idx j lives at partition j%16, column j//16, replicated 8x."""
    T = a.shape[0]
    assert T % 16 == 0
    w = a.reshape(T // 16, 16).T.astype(np.int16)  # [16, T//16]
    return np.tile(w, (8, 1))                       # [128, T//16]


def _chunks(nt: int) -> list[int]:
    k = -(-nt // NB)
    base = nt // k
    out = [base] * k
    for i in range(nt - base * k):
        out[i] += 1
    return out


def _remap_row(n):
    """Global node id -> row in the split-AllGather table layout.
    AG half1 gathers local rows [0,640) of all cores into table rows
    [0, 5120); half2 gathers [640,1280) into [5120, 10240)."""
    c = n // NSL
    o = n % NSL
    return np.where(o < NHALF, c * NHALF + o, NCORES * NHALF + c * NHALF + (o - NHALF))


def build_program(tiles_w: list[int], skip=frozenset(), reps=1, split_ag=True,
                  gmode="chunk", ring_bytes=16384, gbufs=2, nq=2,
                  qmode="gather", pf=0):
    """Build the SPMD Bass program.  tiles_w[w] = number of 128-edge
    tiles in window w (identical across cores, host-padded)."""
    TOT = sum(tiles_w) * 128          # padded edges per core
    NTIL = sum(tiles_w)

    gbufs = max(gbufs, pf + 1)
    # dma_gather is capped at 1024 indices per op (Q7 ucode); a bigger
    # SWDGE ring lets several 1024-desc gathers be in flight so descriptor
    # generation overlaps the previous gather's transfer.
    nc = bacc.Bacc("TRN2", target_bir_lowering=False, debug=False,
                   num_devices=NCORES, dynamic_dma_scratch_size=ring_bytes,
                   num_swdge_queues=nq)

    # ---- I/O ----
    xsl = nc.dram_tensor("xsl", [NSL, NFP], F32, kind="ExternalInput")
    rowi = nc.dram_tensor("rowi", [128, TOT // 16], I16, kind="ExternalInput")
    coli = nc.dram_tensor("coli", [128, TOT // 16], I16, kind="ExternalInput")
    ssp_d = nc.dram_tensor("ssp", [128, NTIL * 128], BF16, kind="ExternalInput")
    sspt_d = nc.dram_tensor("sspt", [128, NTIL * 128], BF16,
                            kind="ExternalInput")
    strow_d = nc.dram_tensor("strow", [1, NSL], F32, kind="ExternalInput")
    w1_d = nc.dram_tensor("w1", [NFP, C], F32, kind="ExternalInput")
    b1_d = nc.dram_tensor("b1", [C], F32, kind="ExternalInput")
    wq_d = nc.dram_tensor("wq", [L, C, C], F32, kind="ExternalInput")
    wkt_d = nc.dram_tensor("wkt", [L, C, C], F32, kind="ExternalInput")
    wv_d = nc.dram_tensor("wv", [L, C, C], F32, kind="ExternalInput")
    bq_d = nc.dram_tensor("bq", [L, C], F32, kind="ExternalInput")
    bv_d = nc.dram_tensor("bv", [L, C], F32, kind="ExternalInput")
    l2w_d = nc.dram_tensor("l2w", [C, DOUT], F32, kind="ExternalInput")
    l2b_d = nc.dram_tensor("l2b", [DOUT], F32, kind="ExternalInput")
    y_d = nc.dram_tensor("y", [NSL, DOUT], F32, kind="ExternalOutput")

    # ---- internal DRAM ----
    xsl_d = nc.dram_tensor("xsl_int", [NSL, C], EDT)     # AG input (x_l slice)
    qsl_d = nc.dram_tensor("qsl_int", [NSL, C], EDT)     # qt table (local cols)
    xf_b = nc.dram_tensor("xf_b", [NP, C], EDT, addr_space="Shared")
    tq_f = nc.dram_tensor("tq_f", [NP, L * C], EDT)      # packed x0..x4 rows

    groups = [list(range(NCORES))]
    HROWS = NCORES * NHALF   # 5120

    with tile.TileContext(nc) as tc:
        with (
            tc.tile_pool(name="const", bufs=1) as cpool,
            tc.tile_pool(name="work", bufs=2) as pool,
            tc.tile_pool(name="gat", bufs=gbufs) as gpool,
            tc.tile_pool(name="psum", bufs=3 if qmode != "pe" else 2,
                         space="PSUM") as psp,
            tc.tile_pool(name="psw", bufs=2, space="PSUM") as pswp,
            tc.tile_pool(name="psq", bufs=2, space="PSUM") as qpsp,
        ):
            # ---------- constants ----------
            ident = cpool.tile([128, 128], F32)
            make_identity(nc, ident[:])

            w1_sb = cpool.tile([NFP, C], F32)
            nc.sync.dma_start(out=w1_sb[:], in_=w1_d[:])
            b1_sb = cpool.tile([C, 1], F32)
            nc.sync.dma_start(out=b1_sb[:], in_=b1_d[:, None])
            l2w_sb = cpool.tile([C, DOUT], F32)
            nc.sync.dma_start(out=l2w_sb[:], in_=l2w_d[:])
            l2b_sb = cpool.tile([1, DOUT], F32)
            nc.sync.dma_start(out=l2b_sb[:], in_=l2b_d[:][None, :])
            ones_row = cpool.tile([1, 128], F32)
            nc.gpsimd.memset(ones_row[:], 1.0)

            rowi_sb = cpool.tile([128, TOT // 16], I16)
            nc.sync.dma_start(out=rowi_sb[:], in_=rowi[:])
            coli_sb = cpool.tile([128, TOT // 16], I16)
            nc.sync.dma_start(out=coli_sb[:], in_=coli[:])
            if gmode == "chunk":
                ssp_sb = cpool.tile([128, NTIL, 128], BF16)
                nc.sync.dma_start(
                    out=ssp_sb[:],
                    in_=ssp_d[:].rearrange("p (t n) -> p t n", t=NTIL))
            MAXT = max(tiles_w)
            strow_sb = cpool.tile([1, NSL], F32)
            nc.sync.dma_start(out=strow_sb[:], in_=strow_d[:])

            xc_all = cpool.tile([128, NSL], F32)    # current x_l, c-major
            qt_all = cpool.tile([128, NW, C], EDT)  # qt, node-major per window

            def ag_table(lslice):
                """Split AllGather of xsl_d into xf_b, then copy into the
                packed table column block for layer-slice `lslice`."""
                if not split_ag:
                    return
                if "cc" not in skip:
                    nc.gpsimd.collective_compute(
                        "AllGather", mybir.AluOpType.bypass,
                        replica_groups=groups,
                        ins=[xsl_d[0:NHALF]], outs=[xf_b[0:HROWS]])
                nc.sync.dma_start(out=tq_f[0:HROWS, lslice * C:(lslice + 1) * C],
                                  in_=xf_b[0:HROWS])

            def ag_table2(lslice):
                if not split_ag:
                    if "cc" not in skip:
                        nc.gpsimd.collective_compute(
                            "AllGather", mybir.AluOpType.bypass,
                            replica_groups=groups,
                            ins=[xsl_d[0:NSL]], outs=[xf_b[0:NP]])
                    nc.sync.dma_start(
                        out=tq_f[:, lslice * C:(lslice + 1) * C], in_=xf_b[:])
                    return
                if "cc" not in skip:
                    nc.gpsimd.collective_compute(
                        "AllGather", mybir.AluOpType.bypass,
                        replica_groups=groups,
                        ins=[xsl_d[NHALF:NSL]], outs=[xf_b[HROWS:NP]])
                nc.sync.dma_start(out=tq_f[HROWS:NP, lslice * C:(lslice + 1) * C],
                                  in_=xf_b[HROWS:NP])

            for _rep in range(reps):
              # ---------- x0 = relu(x @ W1 + b1) ----------
              for w in range(NW):
                  xin = pool.tile([128, NFP], F32, tag="xin")
                  nc.sync.dma_start(out=xin[:],
                                    in_=xsl[w * 128:(w + 1) * 128, :])
                  pxt = psp.tile([NFP, 128], F32, tag="pnt")
                  nc.tensor.transpose(pxt[:], xin[:], ident[:])
                  xt = pool.tile([NFP, 128], F32, tag="xt")
                  nc.scalar.copy(xt[:], pxt[:])
                  pm = psp.tile([128, 128], F32, tag="pm")
                  nc.tensor.matmul(pm[:], lhsT=w1_sb[:], rhs=xt[:],
                                   start=True, stop=True)
                  nc.scalar.activation(
                      out=xc_all[:, w * 128:(w + 1) * 128], in_=pm[:],
                      func=mybir.ActivationFunctionType.Relu,
                      bias=b1_sb[:, 0:1])
                  pnt = psp.tile([128, 128], F32, tag="pnt")
                  nc.tensor.transpose(pnt[:], xc_all[:, w * 128:(w + 1) * 128],
                                      ident[:])
                  xn = pool.tile([128, C], EDT, tag="xn")
                  nc.scalar.copy(xn[:], pnt[:])
                  nc.sync.dma_start(out=xsl_d[w * 128:(w + 1) * 128, :],
                                    in_=xn[:])
                  if w == NW // 2 - 1:
                      ag_table(0)
              ag_table2(0)

              # ---------- layers ----------
              for l in range(L):
                  Lc = l + 1
                  wq_sb = pool.tile([128, 128], F32, tag="wq_sb")
                  nc.sync.dma_start(out=wq_sb[:], in_=wq_d[l])
                  wkt_sb = pool.tile([128, 128], F32, tag="wkt_sb")
                  nc.sync.dma_start(out=wkt_sb[:], in_=wkt_d[l])
                  wv_sb = pool.tile([128, 128], F32, tag="wv_sb")
                  nc.sync.dma_start(out=wv_sb[:], in_=wv_d[l])
                  bq_sb = pool.tile([C, 1], F32, tag="bq_sb")
                  nc.sync.dma_start(out=bq_sb[:], in_=bq_d[l][:, None])
                  bv_row = pool.tile([1, C], F32, tag="bv_row")
                  nc.sync.dma_start(out=bv_row[:], in_=bv_d[l][None, :])

                  # qt = glinT(glin(x_l, Wq)+bq, Wk) / 4, from xc_all (c-major)
                  for ch in range(NW):
                      pq = psp.tile([128, 128], F32, tag="pm")
                      nc.tensor.matmul(pq[:], lhsT=wq_sb[:],
                                       rhs=xc_all[:, ch * 128:(ch + 1) * 128],
                                       start=True, stop=True)
                      qs = pool.tile([128, 128], F32, tag="qs")
                      nc.scalar.activation(
                          out=qs[:], in_=pq[:],
                          func=mybir.ActivationFunctionType.Identity,
                          bias=bq_sb[:, 0:1])
                      pq2 = psp.tile([128, 128], F32, tag="pm")
                      nc.tensor.matmul(pq2[:], lhsT=wkt_sb[:], rhs=qs[:],
                                       start=True, stop=True)
                      qtc = pool.tile([128, 128], F32, tag="qtc")
                      nc.scalar.activation(
                          out=qtc[:], in_=pq2[:],
                          func=mybir.ActivationFunctionType.Copy, scale=0.25)
                      pq3 = psp.tile([128, 128], F32, tag="pnt")
                      nc.tensor.transpose(pq3[:], qtc[:], ident[:])
                      if qmode == "pe":
                          nc.scalar.copy(qt_all[:, ch, :], pq3[:])
                      else:
                          qn = pool.tile([128, C], EDT, tag="xn")
                          nc.scalar.copy(qn[:], pq3[:])
                          nc.sync.dma_start(
                              out=qsl_d[ch * 128:(ch + 1) * 128, :], in_=qn[:])

                  # ---- edge phase + per-window aggregation + dense ----
                  chunk_list = []
                  _t0w = 0
                  for w in range(NW):
                      _tc = 0
                      for nb in _chunks(tiles_w[w]):
                          chunk_list.append((w, _t0w, _tc, nb))
                          _tc += nb
                      _t0w += tiles_w[w]

                  gtiles = {}

                  def issue_gather(i):
                      w, gt0, tcc, nb = chunk_list[i]
                      g0 = gt0 + tcc
                      qg = gpool.tile([128, NB, C], EDT, tag="qg")
                      xga = gpool.tile([128, NB, Lc * C], EDT, tag="xga")
                      gtiles[i] = (qg, xga)
                      if "gather" in skip:
                          nc.vector.memset(qg[:, :nb, :], 0.25)
                          nc.vector.memset(xga[:, :nb, :], 0.25)
                          return
                      if qmode == "pe":
                          sst = gpool.tile([128, NB, 128], BF16, tag="sst")
                          nc.sync.dma_start(
                              out=sst[:, :nb, :],
                              in_=sspt_d[:, g0 * 128:(g0 + nb) * 128].rearrange(
                                  "p (t n) -> p t n", t=nb))
                          for h0 in range(0, nb, 4):
                              hn = min(4, nb - h0)
                              qgp = qpsp.tile([128, 4, C], F32, tag="qgp")
                              for t in range(hn):
                                  nc.tensor.matmul(
                                      qgp[:, t, :], lhsT=sst[:, h0 + t, :],
                                      rhs=qt_all[:, w, :], start=True, stop=True)
                              nc.scalar.copy(qg[:, h0:h0 + hn, :], qgp[:, :hn, :])
                      else:
                          nc.gpsimd.dma_gather(
                              qg[:, :nb, :], qsl_d[:],
                              coli_sb[:, g0 * 8:(g0 + nb) * 8],
                              nb * 128, nb * 128, C, queue_num=nq - 1)
                      nc.gpsimd.dma_gather(
                          xga[:, :nb, :], tq_f[:, :Lc * C],
                          rowi_sb[:, g0 * 8:(g0 + nb) * 8],
                          nb * 128, nb * 128, Lc * C, elem_step=L * C,
                          queue_num=(g0 // NB) % nq if qmode == "pe" else 0)

                  for i in range(min(pf, len(chunk_list))):
                      issue_gather(i)

                  upsw = None
                  first = True
                  for i, (w, gt0, tcc, nb) in enumerate(chunk_list):
                      if pf == 0:
                          issue_gather(i)
                      elif i + pf < len(chunk_list):
                          issue_gather(i + pf)
                      if tcc == 0:
                          upsw = pswp.tile([128, C], F32, tag="acc")
                          first = True
                      wtiles = tiles_w[w]
                      qg, xga = gtiles.pop(i)
                      qg_v = qg[:, :nb, :]
                      xga_v = xga[:, :nb, :]
                      MSG = pool.tile([128, NB, Lc, C], EDT, tag="MSG")
                      if "dve" in skip:
                          nc.vector.memset(MSG[:, :nb, :, :], 0.25)
                      else:
                          xv = xga_v.rearrange("p b (l c) -> p b l c", l=Lc)
                          # P = xga * qg (broadcast over slices; 2x)
                          P = pool.tile([128, NB, Lc, C], EDT, tag="P")
                          nc.vector.tensor_tensor(
                              out=P[:, :nb], in0=xv,
                              in1=qg_v.unsqueeze(2).to_broadcast(
                                  [128, nb, Lc, C]),
                              op=mybir.AluOpType.mult)
                          # tree reduce over CH: 3 bf16 stages + f32 tail
                          Ph = P[:, :nb].rearrange(
                              "p b l (h c) -> p b l h c", h=H)
                          T1 = pool.tile([128, NB, Lc, H, 8], EDT, tag="T1")
                          nc.vector.tensor_tensor(
                              out=T1[:, :nb], in0=Ph[:, :, :, :, 0:8],
                              in1=Ph[:, :, :, :, 8:16],
                              op=mybir.AluOpType.add)
                          T2 = pool.tile([128, NB, Lc, H, 4], EDT, tag="T2")
                          nc.vector.tensor_tensor(
                              out=T2[:, :nb],
                              in0=T1[:, :nb, :, :, 0:4],
                              in1=T1[:, :nb, :, :, 4:8],
                              op=mybir.AluOpType.add)
                          T3 = pool.tile([128, NB, Lc, H, 2], EDT, tag="T3")
                          nc.vector.tensor_tensor(
                              out=T3[:, :nb],
                              in0=T2[:, :nb, :, :, 0:2],
                              in1=T2[:, :nb, :, :, 2:4],
                              op=mybir.AluOpType.add)
                          sc = pool.tile([128, NB, H, Lc], F32, tag="sc")
                          nc.vector.tensor_tensor(
                              out=sc[:, :nb].rearrange("p b h l -> p b l h"),
                              in0=T3[:, :nb, :, :, 0],
                              in1=T3[:, :nb, :, :, 1],
                              op=mybir.AluOpType.add)
                          # softmax over slices
                          ex = pool.tile([128, NB, H, Lc], F32, tag="ex")
                          nc.scalar.activation(
                              out=ex[:, :nb], in_=sc[:, :nb],
                              func=mybir.ActivationFunctionType.Exp)
                          den = pool.tile([128, NB, H], F32, tag="den")
                          nc.vector.reduce_sum(out=den[:, :nb, :],
                                               in_=ex[:, :nb],
                                               axis=mybir.AxisListType.X)
                          rec = pool.tile([128, NB, H], F32, tag="rec")
                          nc.vector.reciprocal(rec[:, :nb, :], den[:, :nb, :])
                          attn = pool.tile([128, NB, Lc, H], F32, tag="attn")
                          nc.vector.tensor_tensor(
                              out=attn[:, :nb].rearrange("p b l h -> p b h l"),
                              in0=ex[:, :nb],
                              in1=rec[:, :nb, :].unsqueeze(3).to_broadcast(
                                  [128, nb, H, Lc]),
                              op=mybir.AluOpType.mult)
                          # EXB = attn broadcast over CH (ACT, bf16 out)
                          EXB = pool.tile([128, NB, Lc, H, CH], EDT, tag="EXB")
                          nc.scalar.activation(
                              out=EXB[:, :nb],
                              in_=attn[:, :nb].unsqueeze(
                                  4).to_broadcast([128, nb, Lc, H, CH]),
                              func=mybir.ActivationFunctionType.Copy)
                          # MSG = xga * EXB (2x)
                          nc.vector.tensor_tensor(
                              out=MSG[:, :nb], in0=xv,
                              in1=EXB[:, :nb].rearrange(
                                  "p b l h c -> p b l (h c)"),
                              op=mybir.AluOpType.mult)
                      if "pe" not in skip:
                          for t in range(nb):
                              for j in range(Lc):
                                  nc.tensor.matmul(
                                      upsw[:], lhsT=ssp_sb[:, gt0 + tcc + t, :],
                                      rhs=MSG[:, t, j, :],
                                      start=first,
                                      stop=(t == nb - 1 and j == Lc - 1 and
                                            tcc + nb >= wtiles))
                                  first = False
                      if tcc + nb < wtiles:
                          continue

                      # ---- dense epilogue for this window ----
                      uw = pool.tile([128, C], F32, tag="uw")
                      nc.scalar.copy(uw[:], upsw[:])
                      put = psp.tile([128, C], F32, tag="pnt")
                      nc.tensor.transpose(put[:], uw[:], ident[:])
                      uc = pool.tile([128, C], F32, tag="uc")
                      nc.scalar.copy(uc[:], put[:])
                      pg = psp.tile([128, C], F32, tag="pm")
                      nc.tensor.matmul(pg[:], lhsT=wv_sb[:], rhs=uc[:],
                                       start=True, stop=False)
                      nc.tensor.matmul(pg[:], lhsT=bv_row[:],
                                       rhs=strow_sb[:, w * 128:(w + 1) * 128],
                                       start=False, stop=True)
                      # xl = relu(...)  (c-major, directly into xc_all)
                      nc.scalar.activation(
                          out=xc_all[:, w * 128:(w + 1) * 128], in_=pg[:],
                          func=mybir.ActivationFunctionType.Relu)
                      if l < L - 1:
                          pnt = psp.tile([128, C], F32, tag="pnt")
                          nc.tensor.transpose(
                              pnt[:], xc_all[:, w * 128:(w + 1) * 128], ident[:])
                          xne = pool.tile([128, C], EDT, tag="xn")
                          nc.scalar.copy(xne[:], pnt[:])
                          nc.sync.dma_start(
                              out=xsl_d[w * 128:(w + 1) * 128, :], in_=xne[:])
                          if w == NW // 2 - 1:
                              ag_table(l + 1)
                  if l < L - 1:
                      ag_table2(l + 1)

              # ---------- output: y = x5 @ l2w + l2b ----------
              for ch in range(NW):
                  py = psp.tile([128, DOUT], F32, tag="pm")
                  nc.tensor.matmul(py[:], lhsT=xc_all[:, ch * 128:(ch + 1) * 128],
                                   rhs=l2w_sb[:], start=True, stop=False)
                  nc.tensor.matmul(py[:], lhsT=ones_row[:], rhs=l2b_sb[:],
                                   start=False, stop=True)
                  ysb = pool.tile([128, DOUT], F32, tag="ysb")
                  nc.scalar.copy(ysb[:], py[:])
                  nc.sync.dma_start(out=y_d[ch * 128:(ch + 1) * 128, :],
                                    in_=ysb[:])

    nc.compile()
    return nc


def _prep_host(x, edge_index, row_sort=False):
    """Shard + sort edges, build per-core index inputs.  Edges are
    grouped per 128-node window; within a window they are sorted by
    source row so the xga gather descriptors hit HBM pages in order."""
    row = np.concatenate([np.asarray(edge_index[0]), np.arange(N)]).astype(np.int64)
    col = np.concatenate([np.asarray(edge_index[1]), np.arange(N)]).astype(np.int64)

    core = col // NSL
    counts = np.zeros((NCORES, NW), dtype=np.int64)
    per_core = []
    for c in range(NCORES):
        m = core == c
        rc, cc = row[m], col[m]
        lw0 = (cc - c * NSL) // 128
        key = (lw0 * NP + _remap_row(rc)) if row_sort else cc
        o = np.argsort(key, kind="stable")
        rc, cc = rc[o], cc[o]
        per_core.append((rc, cc))
        lw = (cc - c * NSL) // 128
        for w in range(NW):
            counts[c, w] = int((lw == w).sum())
    tiles_w = [int(np.ceil(counts[:, w].max() / 128)) for w in range(NW)]
    TOT = sum(tiles_w) * 128

    # gcn norm (host): deg over targets incl self-loops; pad nodes get
    # deg=1 (dis=1) but never appear in any edge, so they contribute 0.
    deg = np.bincount(col, minlength=NP).astype(np.float64)
    deg[N:] = 1.0
    dis = 1.0 / np.sqrt(deg)

    rows_p = np.zeros((NCORES, TOT), dtype=np.int64)     # remapped table rows
    cols_p = np.zeros((NCORES, TOT), dtype=np.int64)     # local col idx
    sval_p = np.zeros((NCORES, TOT), dtype=np.float32)   # dis[row]*dis[col]
    nloc_p = np.zeros((NCORES, TOT), dtype=np.int64)     # col within window
    strow = np.zeros((NCORES, NSL), dtype=np.float32)    # dis_i*segsum(dis[row])
    for c in range(NCORES):
        rc, cc = per_core[c]
        lw = (cc - c * NSL) // 128
        pos = 0
        for w in range(NW):
            m = lw == w
            k = int(m.sum())
            rows_p[c, pos:pos + k] = rc[m]
            cols_p[c, pos:pos + k] = cc[m] - c * NSL
            sval_p[c, pos:pos + k] = (dis[rc[m]] * dis[cc[m]]).astype(np.float32)
            nloc_p[c, pos:pos + k] = cc[m] - c * NSL - w * 128
            pos += tiles_w[w] * 128
        lo = np.zeros(NSL, dtype=np.float64)
        np.add.at(lo, cc - c * NSL, dis[rc])
        strow[c] = (dis[c * NSL:(c + 1) * NSL] * lo).astype(np.float32)

    # S'' selection matrices, host-swizzled to [128, NTIL*128] bf16:
    # ssp[p, t*128+n] = (nloc(e)==n) * sval(e) for edge e = t*128+p,
    # 0 for pad slots (sval=0 there).
    NTIL = TOT // 128
    import ml_dtypes
    ssp = np.zeros((NCORES, 128, NTIL * 128), dtype=ml_dtypes.bfloat16)
    sspt = np.zeros((NCORES, 128, NTIL * 128), dtype=ml_dtypes.bfloat16)
    for c in range(NCORES):
        nl = nloc_p[c].reshape(NTIL, 128)     # [t, p]
        sv = sval_p[c].reshape(NTIL, 128)
        t_i, p_i = np.nonzero(sv != 0.0)
        ssp[c, p_i, t_i * 128 + nl[t_i, p_i]] = sv[t_i, p_i].astype(
            ml_dtypes.bfloat16)
        # raw transposed selection (for the PE qg broadcast):
        # sspt[n, t*128+e] = 1 iff edge (t,e) selects window-node n
        t_a = np.arange(NTIL).repeat(128)
        p_a = np.tile(np.arange(128), NTIL)
        sspt[c, nloc_p[c], t_a * 128 + p_a] = ml_dtypes.bfloat16(1.0)

    return tiles_w, rows_p, cols_p, ssp, strow, sspt


def prepare(inputs):
    return _prepare_impl(inputs)


def _prepare_impl(inputs, split_ag=True):
    x = np.asarray(inputs["x"], dtype=np.float32)
    edge_index = np.asarray(inputs["edge_index"])
    lin1_w = np.asarray(inputs["lin1_w"], dtype=np.float32)
    lin1_b = np.asarray(inputs["lin1_b"], dtype=np.float32)
    Wq = np.asarray(inputs["Wq"], dtype=np.float32)
    bq = np.asarray(inputs["bq"], dtype=np.float32)
    Wk = np.asarray(inputs["Wk"], dtype=np.float32)
    Wv = np.asarray(inputs["Wv"], dtype=np.float32)
    bv = np.asarray(inputs["bv"], dtype=np.float32)
    lin2_w = np.asarray(inputs["lin2_w"], dtype=np.float32)
    lin2_b = np.asarray(inputs["lin2_b"], dtype=np.float32)

    tiles_w, rows_p, cols_p, ssp, strow, sspt = _prep_host(x, edge_index)
    if split_ag:
        rows_p = _remap_row(rows_p)
    nc = build_program(tiles_w, split_ag=split_ag, **BUILD_KW)

    def blockdiag(W):  # W [G, CG, CG] -> [C, C]
        out = np.zeros((C, C), dtype=np.float32)
        for g in range(G):
            out[g * CG:(g + 1) * CG, g * CG:(g + 1) * CG] = W[g]
        return out

    wq_bd = np.stack([blockdiag(Wq[l]) for l in range(L)])
    wkt_bd = np.stack([blockdiag(Wk[l].transpose(0, 2, 1)) for l in range(L)])
    wv_bd = np.stack([blockdiag(Wv[l]) for l in range(L)])

    x_pad = np.zeros((NP, NFP), dtype=np.float32)
    x_pad[:N, :NF] = x
    w1_pad = np.zeros((NFP, C), dtype=np.float32)
    w1_pad[:NF] = lin1_w

    in_maps = []
    for c in range(NCORES):
        in_maps.append({
            "xsl": x_pad[c * NSL:(c + 1) * NSL],
            "rowi": _wrap_idx(rows_p[c]),
            "coli": _wrap_idx(cols_p[c]),
            "ssp": np.ascontiguousarray(ssp[c]),
            "sspt": np.ascontiguousarray(sspt[c]),
            "strow": strow[c][None, :],
            "w1": w1_pad,
            "b1": lin1_b,
            "wq": wq_bd,
            "wkt": wkt_bd,
            "wv": wv_bd,
            "bq": bq,
            "bv": bv,
            "l2w": lin2_w,
            "l2b": lin2_b,
        })

    return nc, in_maps


def assemble(res) -> np.ndarray:
    y = np.concatenate([res.results[c]["y"] for c in range(NCORES)], axis=0)
    return np.ascontiguousarray(y[:N]).astype(np.float32)


def kernel(**inputs) -> np.ndarray:
    nc, in_maps = _prepare_impl(inputs)
    res = run_bass_kernel_spmd(nc, in_maps, list(range(NCORES)))
    global LAST_RESULTS
    LAST_RESULTS = res
    return assemble(res)


LAST_RESULTS = None
BUILD_KW = {}


if __name__ == "__main__":
    import reference
    inp = {k: np.asarray(v) for k, v in reference.setup_inputs().items()}
    out = kernel(**inp)
    print(out.shape, out.dtype)



# revision 6
# speedup vs baseline: 1.3996x; 1.3996x over previous
"""Trainium2 Bass kernel v2 for the DNA GNN (nn_DNA_65360812310552).

Changes vs v1:
  - Incremental gather: layer l gathers ONLY slice l (256B/edge) from a
    per-layer AG table tq2[l]; slices 0..l-1 are re-read sequentially
    (HWDGE) from a per-edge DRAM cache written at their own layer.
  - qg via PE selection matmuls from SBUF-resident qt_all (no qg gather).
  - Per-slice DVE ops with contiguous operands (2x bf16 mode).
  - Layer 0 shortcut: softmax over 1 slice == 1, so layer 0 is just
    gather + segment-sum (no qt phase, no scores).
  - Host-side node relabeling balances per-window edge counts.
"""

import numpy as np

import concourse.bacc as bacc
import concourse.bass as bass
import concourse.mybir as mybir
import concourse.tile as tile
from concourse.bass_utils import run_bass_kernel_spmd
from concourse.masks import make_identity

N = 10000
E = 160000
C = 128
H = 8
CH = 16
G = 16
CG = 8
L = 5
NF = 14
NFP = 16
DOUT = 16
NCORES = 8

NP = 10240
NSL = NP // NCORES   # 1280
NW = NSL // 128      # 10
NHALF = NSL // 2     # 640
NB = 8

F32 = mybir.dt.float32
BF16 = mybir.dt.bfloat16
I16 = mybir.dt.int16
EDT = mybir.dt.bfloat16


def _wrap_idx(a: np.ndarray) -> np.ndarray:
    T = a.shape[0]
    assert T % 16 == 0
    w = a.reshape(T // 16, 16).T.astype(np.int16)
    return np.tile(w, (8, 1))


def _chunks(nt: int) -> list[int]:
    k = -(-nt // NB)
    base = nt // k
    out = [base] * k
    for i in range(nt - base * k):
        out[i] += 1
    return out


def _remap_row(n):
    c = n // NSL
    o = n % NSL
    return np.where(o < NHALF, c * NHALF + o, NCORES * NHALF + c * NHALF + (o - NHALF))


def build_program(tiles_w: list[int], skip=frozenset(), reps=1,
                  gbufs=3, nq=2, pf=1, ring_bytes=16384):
    TOT = sum(tiles_w) * 128
    NTIL = sum(tiles_w)

    gbufs = max(gbufs, pf + 1)
    nc = bacc.Bacc("TRN2", target_bir_lowering=False, debug=False,
                   num_devices=NCORES, dynamic_dma_scratch_size=ring_bytes,
                   num_swdge_queues=nq)

    # ---- I/O ----
    xsl = nc.dram_tensor("xsl", [NSL, NFP], F32, kind="ExternalInput")
    rowi = nc.dram_tensor("rowi", [128, TOT // 16], I16, kind="ExternalInput")
    ssp_d = nc.dram_tensor("ssp", [128, NTIL * 128], BF16, kind="ExternalInput")
    sspt_d = nc.dram_tensor("sspt", [128, NTIL * 128], BF16,
                            kind="ExternalInput")
    strow_d = nc.dram_tensor("strow", [1, NSL], F32, kind="ExternalInput")
    w1_d = nc.dram_tensor("w1", [NFP, C], F32, kind="ExternalInput")
    b1_d = nc.dram_tensor("b1", [C], F32, kind="ExternalInput")
    wq_d = nc.dram_tensor("wq", [L, C, C], F32, kind="ExternalInput")
    wkt_d = nc.dram_tensor("wkt", [L, C, C], F32, kind="ExternalInput")
    wv_d = nc.dram_tensor("wv", [L, C, C], F32, kind="ExternalInput")
    bq_d = nc.dram_tensor("bq", [L, C], F32, kind="ExternalInput")
    bv_d = nc.dram_tensor("bv", [L, C], F32, kind="ExternalInput")
    l2w_d = nc.dram_tensor("l2w", [C, DOUT], F32, kind="ExternalInput")
    l2b_d = nc.dram_tensor("l2b", [DOUT], F32, kind="ExternalInput")
    y_d = nc.dram_tensor("y", [NSL, DOUT], F32, kind="ExternalOutput")

    # ---- internal DRAM ----
    xsl_d = nc.dram_tensor("xsl_int", [NSL, C], EDT)          # AG input
    tq2 = nc.dram_tensor("tq2", [L, NP, C], EDT, addr_space="Shared")
    cache_d = nc.dram_tensor("cache", [L - 1, 128, NTIL, C], EDT)

    groups = [list(range(NCORES))]
    HROWS = NCORES * NHALF

    with tile.TileContext(nc) as tc:
        with (
            tc.tile_pool(name="const", bufs=1) as cpool,
            tc.tile_pool(name="work", bufs=2) as pool,
            tc.tile_pool(name="gat", bufs=gbufs) as gpool,
            tc.tile_pool(name="psum", bufs=2, space="PSUM") as psp,
            tc.tile_pool(name="psw", bufs=2, space="PSUM") as pswp,
            tc.tile_pool(name="psq", bufs=2, space="PSUM") as qpsp,
        ):
            ident = cpool.tile([128, 128], F32)
            make_identity(nc, ident[:])

            w1_sb = cpool.tile([NFP, C], F32)
            nc.sync.dma_start(out=w1_sb[:], in_=w1_d[:])
            b1_sb = cpool.tile([C, 1], F32)
            nc.sync.dma_start(out=b1_sb[:], in_=b1_d[:, None])
            l2w_sb = cpool.tile([C, DOUT], F32)
            nc.sync.dma_start(out=l2w_sb[:], in_=l2w_d[:])
            l2b_sb = cpool.tile([1, DOUT], F32)
            nc.sync.dma_start(out=l2b_sb[:], in_=l2b_d[:][None, :])
            ones_row = cpool.tile([1, 128], F32)
            nc.gpsimd.memset(ones_row[:], 1.0)

            rowi_sb = cpool.tile([128, TOT // 16], I16)
            nc.sync.dma_start(out=rowi_sb[:], in_=rowi[:])
            ssp_sb = cpool.tile([128, NTIL, 128], BF16)
            nc.sync.dma_start(
                out=ssp_sb[:],
                in_=ssp_d[:].rearrange("p (t n) -> p t n", t=NTIL))
            strow_sb = cpool.tile([1, NSL], F32)
            nc.sync.dma_start(out=strow_sb[:], in_=strow_d[:])

            xc_all = cpool.tile([128, NSL], F32)    # current x_l, c-major
            # qt per window, node-major; two buffers alternated by layer so
            # layer l+1's qt phase overlaps layer l's last qg reads
            qt_a = cpool.tile([128, NW, C], EDT)
            qt_b = cpool.tile([128, NW, C], EDT)
            qt_bufs = [qt_a, qt_b]

            def ag_half1(lslice):
                if "cc" not in skip:
                    nc.gpsimd.collective_compute(
                        "AllGather", mybir.AluOpType.bypass,
                        replica_groups=groups,
                        ins=[xsl_d[0:NHALF]], outs=[tq2[lslice][0:HROWS]])

            def ag_half2(lslice):
                if "cc" not in skip:
                    nc.gpsimd.collective_compute(
                        "AllGather", mybir.AluOpType.bypass,
                        replica_groups=groups,
                        ins=[xsl_d[NHALF:NSL]], outs=[tq2[lslice][HROWS:NP]])

            for _rep in range(reps):
              # ---------- x0 = relu(x @ W1 + b1) ----------
              for w in range(NW):
                  xin = pool.tile([128, NFP], F32, tag="xin")
                  nc.sync.dma_start(out=xin[:],
                                    in_=xsl[w * 128:(w + 1) * 128, :])
                  pxt = psp.tile([NFP, 128], F32, tag="pnt")
                  nc.tensor.transpose(pxt[:], xin[:], ident[:])
                  xt = pool.tile([NFP, 128], F32, tag="xt")
                  nc.scalar.copy(xt[:], pxt[:])
                  pm = psp.tile([128, 128], F32, tag="pm")
                  nc.tensor.matmul(pm[:], lhsT=w1_sb[:], rhs=xt[:],
                                   start=True, stop=True)
                  nc.scalar.activation(
                      out=xc_all[:, w * 128:(w + 1) * 128], in_=pm[:],
                      func=mybir.ActivationFunctionType.Relu,
                      bias=b1_sb[:, 0:1])
                  pnt = psp.tile([128, 128], F32, tag="pnt")
                  nc.tensor.transpose(pnt[:], xc_all[:, w * 128:(w + 1) * 128],
                                      ident[:])
                  xn = pool.tile([128, C], EDT, tag="xn")
                  nc.scalar.copy(xn[:], pnt[:])
                  nc.sync.dma_start(out=xsl_d[w * 128:(w + 1) * 128, :],
                                    in_=xn[:])
                  if w == NW // 2 - 1:
                      ag_half1(0)
              ag_half2(0)

              # ---------- layers ----------
              for l in range(L):
                  Lc = l + 1
                  wv_sb = pool.tile([128, 128], F32, tag="wv_sb")
                  nc.sync.dma_start(out=wv_sb[:], in_=wv_d[l])
                  bv_row = pool.tile([1, C], F32, tag="bv_row")
                  nc.sync.dma_start(out=bv_row[:], in_=bv_d[l][None, :])

                  qt_all = qt_bufs[l % 2]
                  if l > 0:
                      wq_sb = pool.tile([128, 128], F32, tag="wq_sb")
                      nc.sync.dma_start(out=wq_sb[:], in_=wq_d[l])
                      wkt_sb = pool.tile([128, 128], F32, tag="wkt_sb")
                      nc.sync.dma_start(out=wkt_sb[:], in_=wkt_d[l])
                      bq_sb = pool.tile([C, 1], F32, tag="bq_sb")
                      nc.sync.dma_start(out=bq_sb[:], in_=bq_d[l][:, None])
                      # qt = glinT(glin(x_l, Wq)+bq, Wk) / 4  (node-major)
                      for ch in range(NW):
                          pq = psp.tile([128, 128], F32, tag="pm")
                          nc.tensor.matmul(
                              pq[:], lhsT=wq_sb[:],
                              rhs=xc_all[:, ch * 128:(ch + 1) * 128],
                              start=True, stop=True)
                          qs = pool.tile([128, 128], F32, tag="qs")
                          nc.scalar.activation(
                              out=qs[:], in_=pq[:],
                              func=mybir.ActivationFunctionType.Identity,
                              bias=bq_sb[:, 0:1])
                          pq2 = psp.tile([128, 128], F32, tag="pm")
                          nc.tensor.matmul(pq2[:], lhsT=wkt_sb[:], rhs=qs[:],
                                           start=True, stop=True)
                          qtc = pool.tile([128, 128], F32, tag="qtc")
                          nc.scalar.activation(
                              out=qtc[:], in_=pq2[:],
                              func=mybir.ActivationFunctionType.Copy,
                              scale=0.25)
                          pq3 = psp.tile([128, 128], F32, tag="pnt")
                          nc.tensor.transpose(pq3[:], qtc[:], ident[:])
                          nc.scalar.copy(qt_all[:, ch, :], pq3[:])

                  chunk_list = []
                  _t0w = 0
                  for w in range(NW):
                      _tc = 0
                      for nb in _chunks(tiles_w[w]):
                          chunk_list.append((w, _t0w, _tc, nb))
                          _tc += nb
                      _t0w += tiles_w[w]

                  gtiles = {}

                  def issue_gather(i):
                      w, gt0, tcc, nb = chunk_list[i]
                      g0 = gt0 + tcc
                      xgf = gpool.tile([128, NB, C], EDT, tag="xgf")
                      xgc = None
                      sst = None
                      if l > 0:
                          xgc = gpool.tile([128, L - 1, NB, C], EDT,
                                           tag="xgc")
                          sst = gpool.tile([128, NB, 128], BF16, tag="sst")
                          nc.sync.dma_start(
                              out=sst[:, :nb, :],
                              in_=sspt_d[:, g0 * 128:(g0 + nb) * 128].rearrange(
                                  "p (t n) -> p t n", t=nb))
                      gtiles[i] = (xgf, xgc, sst)
                      if "gather" in skip:
                          nc.vector.memset(xgf[:, :nb, :], 0.25)
                          if xgc is not None:
                              nc.vector.memset(xgc[:, :l, :nb, :], 0.25)
                          return
                      nc.gpsimd.dma_gather(
                          xgf[:, :nb, :], tq2[l],
                          rowi_sb[:, g0 * 8:(g0 + nb) * 8],
                          nb * 128, nb * 128, C,
                          queue_num=i % nq)
                      if l < L - 1:
                          nc.sync.dma_start(
                              out=cache_d[l, :, g0:g0 + nb, :],
                              in_=xgf[:, :nb, :])
                      if l > 0:
                          nc.sync.dma_start(
                              out=xgc[:, :l, :nb, :],
                              in_=cache_d[0:l, :, g0:g0 + nb, :].rearrange(
                                  "j p t c -> p j t c"))

                  for i in range(min(pf, len(chunk_list))):
                      issue_gather(i)

                  upsw = None
                  first = True
                  for i, (w, gt0, tcc, nb) in enumerate(chunk_list):
                      if pf == 0:
                          issue_gather(i)
                      elif i + pf < len(chunk_list):
                          issue_gather(i + pf)
                      if tcc == 0:
                          upsw = pswp.tile([128, C], F32, tag="acc")
                          first = True
                      wtiles = tiles_w[w]
                      xgf, xgc, sst = gtiles.pop(i)

                      def xga_j(j, t=None):
                          src = xgf if j == l else xgc[:, j]
                          if t is None:
                              return src[:, :nb, :]
                          return src[:, t, :]

                      if l == 0:
                          # attn == 1: msg = x0[row]; segment-sum directly
                          if "pe" not in skip:
                              for t in range(nb):
                                  nc.tensor.matmul(
                                      upsw[:],
                                      lhsT=ssp_sb[:, gt0 + tcc + t, :],
                                      rhs=xgf[:, t, :],
                                      start=first,
                                      stop=(t == nb - 1 and
                                            tcc + nb >= wtiles))
                                  first = False
                      else:
                          # qg via PE selection from qt_all
                          qg = pool.tile([128, NB, C], EDT, tag="qg")
                          if "dve" in skip:
                              nc.vector.memset(qg[:, :nb, :], 0.25)
                          else:
                              for h0 in range(0, nb, 4):
                                  hn = min(4, nb - h0)
                                  qgp = qpsp.tile([128, 4, C], F32, tag="qgp")
                                  for t in range(hn):
                                      nc.tensor.matmul(
                                          qgp[:, t, :],
                                          lhsT=sst[:, h0 + t, :],
                                          rhs=qt_all[:, w, :],
                                          start=True, stop=True)
                                  nc.scalar.copy(qg[:, h0:h0 + hn, :],
                                                 qgp[:, :hn, :])

                          MSG = pool.tile([128, Lc, NB, C], EDT, tag="MSG")
                          if "dve" in skip:
                              nc.vector.memset(MSG[:, :, :nb, :], 0.25)
                          else:
                              P = pool.tile([128, Lc, NB, C], EDT, tag="P")
                              for j in range(Lc):
                                  nc.vector.tensor_tensor(
                                      out=P[:, j, :nb, :], in0=xga_j(j),
                                      in1=qg[:, :nb, :],
                                      op=mybir.AluOpType.mult)
                              Ph = P.rearrange("p l b (h c) -> p l b h c",
                                               h=H)
                              T1 = pool.tile([128, Lc, NB, H, 8], EDT,
                                             tag="T1")
                              nc.vector.tensor_tensor(
                                  out=T1[:, :, :nb],
                                  in0=Ph[:, :, :nb, :, 0:8],
                                  in1=Ph[:, :, :nb, :, 8:16],
                                  op=mybir.AluOpType.add)
                              T2 = pool.tile([128, Lc, NB, H, 4], EDT,
                                             tag="T2")
                              nc.vector.tensor_tensor(
                                  out=T2[:, :, :nb],
                                  in0=T1[:, :, :nb, :, 0:4],
                                  in1=T1[:, :, :nb, :, 4:8],
                                  op=mybir.AluOpType.add)
                              T3 = pool.tile([128, Lc, NB, H, 2], EDT,
                                             tag="T3")
                              nc.vector.tensor_tensor(
                                  out=T3[:, :, :nb],
                                  in0=T2[:, :, :nb, :, 0:2],
                                  in1=T2[:, :, :nb, :, 2:4],
                                  op=mybir.AluOpType.add)
                              sc = pool.tile([128, NB, H, Lc], F32, tag="sc")
                              nc.vector.tensor_tensor(
                                  out=sc[:, :nb].rearrange(
                                      "p b h l -> p l b h"),
                                  in0=T3[:, :, :nb, :, 0],
                                  in1=T3[:, :, :nb, :, 1],
                                  op=mybir.AluOpType.add)
                              ex = pool.tile([128, NB, H, Lc], F32, tag="ex")
                              nc.scalar.activation(
                                  out=ex[:, :nb], in_=sc[:, :nb],
                                  func=mybir.ActivationFunctionType.Exp)
                              den = pool.tile([128, NB, H], F32, tag="den")
                              nc.vector.reduce_sum(out=den[:, :nb, :],
                                                   in_=ex[:, :nb],
                                                   axis=mybir.AxisListType.X)
                              rec = pool.tile([128, NB, H], F32, tag="rec")
                              nc.vector.reciprocal(rec[:, :nb, :],
                                                   den[:, :nb, :])
                              attn = pool.tile([128, NB, H, Lc], F32,
                                               tag="attn")
                              nc.vector.tensor_tensor(
                                  out=attn[:, :nb],
                                  in0=ex[:, :nb],
                                  in1=rec[:, :nb, :].unsqueeze(
                                      3).to_broadcast([128, nb, H, Lc]),
                                  op=mybir.AluOpType.mult)
                              EXB = pool.tile([128, Lc, NB, H, CH], EDT,
                                              tag="EXB")
                              nc.scalar.activation(
                                  out=EXB[:, :, :nb],
                                  in_=attn[:, :nb].rearrange(
                                      "p b h l -> p l b h").unsqueeze(
                                      4).to_broadcast([128, Lc, nb, H, CH]),
                                  func=mybir.ActivationFunctionType.Copy)
                              EXBf = EXB.rearrange("p l b h c -> p l b (h c)")
                              for j in range(Lc):
                                  nc.vector.tensor_tensor(
                                      out=MSG[:, j, :nb, :], in0=xga_j(j),
                                      in1=EXBf[:, j, :nb, :],
                                      op=mybir.AluOpType.mult)
                          if "pe" not in skip:
                              for t in range(nb):
                                  for j in range(Lc):
                                      nc.tensor.matmul(
                                          upsw[:],
                                          lhsT=ssp_sb[:, gt0 + tcc + t, :],
                                          rhs=MSG[:, j, t, :],
                                          start=first,
                                          stop=(t == nb - 1 and j == Lc - 1
                                                and tcc + nb >= wtiles))
                                      first = False
                      if tcc + nb < wtiles:
                          continue

                      # ---- dense epilogue for this window ----
                      uw = pool.tile([128, C], F32, tag="uw")
                      nc.scalar.copy(uw[:], upsw[:])
                      put = psp.tile([128, C], F32, tag="pnt")
                      nc.tensor.transpose(put[:], uw[:], ident[:])
                      uc = pool.tile([128, C], F32, tag="uc")
                      nc.scalar.copy(uc[:], put[:])
                      pg = psp.tile([128, C], F32, tag="pm")
                      nc.tensor.matmul(pg[:], lhsT=wv_sb[:], rhs=uc[:],
                                       start=True, stop=False)
                      nc.tensor.matmul(pg[:], lhsT=bv_row[:],
                                       rhs=strow_sb[:, w * 128:(w + 1) * 128],
                                       start=False, stop=True)
                      nc.scalar.activation(
                          out=xc_all[:, w * 128:(w + 1) * 128], in_=pg[:],
                          func=mybir.ActivationFunctionType.Relu)
                      if l < L - 1:
                          pnt = psp.tile([128, C], F32, tag="pnt")
                          nc.tensor.transpose(
                              pnt[:], xc_all[:, w * 128:(w + 1) * 128],
                              ident[:])
                          xne = pool.tile([128, C], EDT, tag="xn")
                          nc.scalar.copy(xne[:], pnt[:])
                          nc.sync.dma_start(
                              out=xsl_d[w * 128:(w + 1) * 128, :], in_=xne[:])
                          if w == NW // 2 - 1:
                              ag_half1(l + 1)
                  if l < L - 1:
                      ag_half2(l + 1)

              # ---------- output: y = x5 @ l2w + l2b ----------
              for ch in range(NW):
                  py = psp.tile([128, DOUT], F32, tag="pm")
                  nc.tensor.matmul(py[:],
                                   lhsT=xc_all[:, ch * 128:(ch + 1) * 128],
                                   rhs=l2w_sb[:], start=True, stop=False)
                  nc.tensor.matmul(py[:], lhsT=ones_row[:], rhs=l2b_sb[:],
                                   start=False, stop=True)
                  ysb = pool.tile([128, DOUT], F32, tag="ysb")
                  nc.scalar.copy(ysb[:], py[:])
                  nc.sync.dma_start(out=y_d[ch * 128:(ch + 1) * 128, :],
                                    in_=ysb[:])

    nc.compile()
    return nc


def _balance_perm(edge_index):
    """Relabel nodes so each 128-node window has a balanced edge count.
    Returns perm (old id -> new id) over NP padded ids."""
    col = np.asarray(edge_index[1])
    deg = np.bincount(col, minlength=N).astype(np.int64) + 1  # + self-loop
    order = np.argsort(-deg, kind="stable")
    nwin = NP // 128
    wsum = np.zeros(nwin, dtype=np.int64)
    wcnt = np.zeros(nwin, dtype=np.int64)
    slot = np.zeros(N, dtype=np.int64)
    import heapq
    heap = [(0, 0, w) for w in range(nwin)]
    heapq.heapify(heap)
    for nid in order:
        while True:
            s, c, w = heapq.heappop(heap)
            if c < 128:
                break
        slot[nid] = w * 128 + c
        heapq.heappush(heap, (s + deg[nid], c + 1, w))
    perm = np.zeros(NP, dtype=np.int64)
    perm[:N] = slot
    # pad ids fill the remaining slots
    used = np.zeros(NP, dtype=bool)
    used[slot] = True
    perm[N:] = np.nonzero(~used)[0]
    return perm


def _prep_host(x, edge_index, row_sort=True, balance=True):
    row0 = np.concatenate([np.asarray(edge_index[0]), np.arange(N)]).astype(np.int64)
    col0 = np.concatenate([np.asarray(edge_index[1]), np.arange(N)]).astype(np.int64)
    if balance:
        perm = _balance_perm(edge_index)
        row = perm[row0]
        col = perm[col0]
    else:
        perm = np.arange(NP, dtype=np.int64)
        row, col = row0, col0

    core = col // NSL
    counts = np.zeros((NCORES, NW), dtype=np.int64)
    per_core = []
    for c in range(NCORES):
        m = core == c
        rc, cc = row[m], col[m]
        lw0 = (cc - c * NSL) // 128
        key = (lw0 * NP + _remap_row(rc)) if row_sort else cc
        o = np.argsort(key, kind="stable")
        rc, cc = rc[o], cc[o]
        per_core.append((rc, cc))
        lw = (cc - c * NSL) // 128
        for w in range(NW):
            counts[c, w] = int((lw == w).sum())
    tiles_w = [int(np.ceil(counts[:, w].max() / 128)) for w in range(NW)]
    TOT = sum(tiles_w) * 128

    deg = np.bincount(col, minlength=NP).astype(np.float64)
    deg[deg == 0] = 1.0
    dis = 1.0 / np.sqrt(deg)

    rows_p = np.zeros((NCORES, TOT), dtype=np.int64)
    sval_p = np.zeros((NCORES, TOT), dtype=np.float32)
    nloc_p = np.zeros((NCORES, TOT), dtype=np.int64)
    strow = np.zeros((NCORES, NSL), dtype=np.float32)
    for c in range(NCORES):
        rc, cc = per_core[c]
        lw = (cc - c * NSL) // 128
        pos = 0
        for w in range(NW):
            m = lw == w
            k = int(m.sum())
            rows_p[c, pos:pos + k] = rc[m]
            sval_p[c, pos:pos + k] = (dis[rc[m]] * dis[cc[m]]).astype(np.float32)
            nloc_p[c, pos:pos + k] = cc[m] - c * NSL - w * 128
            pos += tiles_w[w] * 128
        lo = np.zeros(NSL, dtype=np.float64)
        np.add.at(lo, cc - c * NSL, dis[rc])
        strow[c] = (dis[c * NSL:(c + 1) * NSL] * lo).astype(np.float32)

    NTIL = TOT // 128
    import ml_dtypes
    ssp = np.zeros((NCORES, 128, NTIL * 128), dtype=ml_dtypes.bfloat16)
    sspt = np.zeros((NCORES, 128, NTIL * 128), dtype=ml_dtypes.bfloat16)
    for c in range(NCORES):
        nl = nloc_p[c].reshape(NTIL, 128)
        sv = sval_p[c].reshape(NTIL, 128)
        t_i, p_i = np.nonzero(sv != 0.0)
        ssp[c, p_i, t_i * 128 + nl[t_i, p_i]] = sv[t_i, p_i].astype(
            ml_dtypes.bfloat16)
        t_a = np.arange(NTIL).repeat(128)
        p_a = np.tile(np.arange(128), NTIL)
        sspt[c, nloc_p[c], t_a * 128 + p_a] = ml_dtypes.bfloat16(1.0)

    return tiles_w, rows_p, ssp, strow, sspt, perm


def _prepare_impl(inputs, build=True, row_sort=True, balance=True):
    x = np.asarray(inputs["x"], dtype=np.float32)
    edge_index = np.asarray(inputs["edge_index"])
    lin1_w = np.asarray(inputs["lin1_w"], dtype=np.float32)
    lin1_b = np.asarray(inputs["lin1_b"], dtype=np.float32)
    Wq = np.asarray(inputs["Wq"], dtype=np.float32)
    bq = np.asarray(inputs["bq"], dtype=np.float32)
    Wk = np.asarray(inputs["Wk"], dtype=np.float32)
    Wv = np.asarray(inputs["Wv"], dtype=np.float32)
    bv = np.asarray(inputs["bv"], dtype=np.float32)
    lin2_w = np.asarray(inputs["lin2_w"], dtype=np.float32)
    lin2_b = np.asarray(inputs["lin2_b"], dtype=np.float32)

    tiles_w, rows_p, ssp, strow, sspt, perm = _prep_host(
        x, edge_index, row_sort=row_sort, balance=balance)
    rows_p = _remap_row(rows_p)
    nc = build_program(tiles_w, **BUILD_KW) if build else None

    def blockdiag(W):
        out = np.zeros((C, C), dtype=np.float32)
        for g in range(G):
            out[g * CG:(g + 1) * CG, g * CG:(g + 1) * CG] = W[g]
        return out

    wq_bd = np.stack([blockdiag(Wq[l]) for l in range(L)])
    wkt_bd = np.stack([blockdiag(Wk[l].transpose(0, 2, 1)) for l in range(L)])
    wv_bd = np.stack([blockdiag(Wv[l]) for l in range(L)])

    # x rows permuted into the new node order
    x_pad = np.zeros((NP, NFP), dtype=np.float32)
    x_pad[perm[:N], :NF] = x

    w1_pad = np.zeros((NFP, C), dtype=np.float32)
    w1_pad[:NF] = lin1_w

    in_maps = []
    for c in range(NCORES):
        in_maps.append({
            "xsl": x_pad[c * NSL:(c + 1) * NSL],
            "rowi": _wrap_idx(rows_p[c]),
            "ssp": np.ascontiguousarray(ssp[c]),
            "sspt": np.ascontiguousarray(sspt[c]),
            "strow": strow[c][None, :],
            "w1": w1_pad,
            "b1": lin1_b,
            "wq": wq_bd,
            "wkt": wkt_bd,
            "wv": wv_bd,
            "bq": bq,
            "bv": bv,
            "l2w": lin2_w,
            "l2b": lin2_b,
        })

    return nc, in_maps, perm


def assemble(res, perm) -> np.ndarray:
    y = np.concatenate([res.results[c]["y"] for c in range(NCORES)], axis=0)
    return np.ascontiguousarray(y[perm[:N]]).astype(np.float32)


def kernel(**inputs) -> np.ndarray:
    nc, in_maps, perm = _prepare_impl(inputs)
    res = run_bass_kernel_spmd(nc, in_maps, list(range(NCORES)))
    global LAST_RESULTS
    LAST_RESULTS = res
    return assemble(res, perm)


LAST_RESULTS = None
BUILD_KW = {}


if __name__ == "__main__":
    import reference
    inp = {k: np.asarray(v) for k, v in reference.setup_inputs().items()}
    out = kernel(**inp)
    print(out.shape, out.dtype)
